# revision 28
# baseline (speedup 1.0000x reference)
"""Trainium2 Bass kernel for nn_AttentionalCopula (sparse_attention).

Sharding: data-parallel over batch (B=8 -> 8 cores); per-head K/V MLP stacks
computed locally per core (each core owns one batch, all heads), so no
collectives are needed.  Weights are replicated.

Per-core pipeline:
  stage A: 32 (l, kv, h) MLP chains  [2048,257] -> 256 -> 256 -> 64
           computed with transposed activations (features on partitions) and
           a row-major final projection written via staging -> DRAM.
  gather:  indirect-DMA row gathers of K/V at (s, t(m,s,n)) positions.
  attn:    DVE/ACT vector math (m on partitions), softmax-over-2 == sigmoid.
  FF/DE:   small matmuls with PE transposes at the LN -> FF boundaries.
"""

from contextlib import ExitStack

import ml_dtypes
import numpy as np

import concourse.bass as bass
import concourse.mybir as mybir
import concourse.tile as tile
from concourse import bacc
from concourse.bass_utils import run_bass_kernel_spmd
from concourse.masks import make_identity

# problem constants (hardcoded per harness contract)
B, S, T, D = 8, 8, 256, 256
H, DK = 8, 64
HD = H * DK            # 512
L = 2
F = 256
R = 512
M = 128
EPS = 1e-5
NROW = S * T           # 2048
NCORES = 8
P = 128

F32 = mybir.dt.float32
F32R = mybir.dt.float32r
BF16 = mybir.dt.bfloat16
I32 = mybir.dt.int32

KV_DT = BF16           # staging / DRAM K/V / gathered tiles

NCHAIN = 2 * L * H     # 32 chains: c = (l*2 + kv)*8 + h

# BCAST column layout (free-dim biases / LN params, replicated across partitions)
BC_DSB = 0                      # ds_b                  [512]
BC_B3V8 = 512                   # S * b3 of V chains, per l   [2*512]
BC_FFB2 = BC_B3V8 + L * HD      # ffb2 per l            [2*512]
BC_DEB3 = BC_FFB2 + L * HD      # deb3                  [512]
BC_LN1G = BC_DEB3 + R           # ln1_g per l           [2*512]
BC_LN1B = BC_LN1G + L * HD
BC_LN2G = BC_LN1B + L * HD
BC_LN2B = BC_LN2G + L * HD
BC_COLS = BC_LN2B + L * HD      # total

# PBIAS column layout (per-partition biases)
PB_B1 = 0                       # b1: 2 cols per chain (f-tile)    [64]
PB_B2 = PB_B1 + 2 * NCHAIN      # b2: 2 cols per chain             [64]
PB_FFB1 = PB_B2 + 2 * NCHAIN    # ffb1: 4 cols per l               [8]
PB_DEB1 = PB_FFB1 + 4 * L       # deb1: 2 cols                     [2]
PB_DEB2 = PB_DEB1 + 2           # deb2: 2 cols                     [2]
PB_W1L = PB_DEB2 + 2            # w1 last row (u weight): 2 cols per chain [64]
PB_COLS = PB_W1L + 2 * NCHAIN


def _emit(nc, tc, tensors):
    XT, XTU, PREDT, DSW, W1, W1L, W2, W3, FFW1, FFW2, DEW1, DEW2, DEW3, \
        PBIAS, BCAST, IDX, OUT, KD0, VD0, KD1, VD1 = tensors
    KVD = [[KD0, VD0], [KD1, VD1]]

    with ExitStack() as ctx:
        cp = ctx.enter_context(tc.tile_pool(name="const", bufs=1))
        wp = ctx.enter_context(tc.tile_pool(name="w", bufs=2))
        hp = ctx.enter_context(tc.tile_pool(name="h", bufs=3))
        sp = ctx.enter_context(tc.tile_pool(name="stag", bufs=2))
        fp = ctx.enter_context(tc.tile_pool(name="ffw", bufs=1))
        gp = ctx.enter_context(tc.tile_pool(name="gath", bufs=1))
        ap = ctx.enter_context(tc.tile_pool(name="attn", bufs=1))
        pp = ctx.enter_context(tc.tile_pool(name="ps", bufs=6, space="PSUM"))
        pa = ctx.enter_context(tc.tile_pool(name="psa", bufs=2, space="PSUM"))

        # ---- resident loads (ds-matmul inputs first) ----
        predt = cp.tile([P, 2, M], F32R)
        nc.sync.dma_start(predt[:], PREDT.ap())
        dsw = cp.tile([P, 2, HD], F32R)
        nc.sync.dma_start(dsw[:], DSW.ap())
        xt = cp.tile([P, 2, NROW], BF16)
        for cc in range(4):
            csl = slice(cc * 512, (cc + 1) * 512)
            nc.sync.dma_start(xt[:, :, csl], XT.ap()[:, :, csl])
        xtu = cp.tile([1, NROW], BF16)
        nc.sync.dma_start(xtu[:], XTU.ap())
        dew1 = cp.tile([P, 4, F], F32R)
        nc.gpsimd.dma_start(dew1[:], DEW1.ap())
        dew2 = cp.tile([P, 2, F], F32R)
        nc.gpsimd.dma_start(dew2[:], DEW2.ap())
        dew3 = cp.tile([P, 2, R], F32R)
        nc.gpsimd.dma_start(dew3[:], DEW3.ap())
        pbias = cp.tile([P, PB_COLS], F32)
        nc.sync.dma_start(pbias[:], PBIAS.ap())
        bcast = cp.tile([P, BC_COLS], F32)
        nc.gpsimd.dma_start(bcast[:], BCAST.ap())
        idx = cp.tile([P, 16], I32)
        nc.sync.dma_start(idx[:], IDX.ap())
        ident = cp.tile([P, P], F32)
        make_identity(nc, ident[:])
        epst = cp.tile([P, 1], F32)
        nc.vector.memset(epst[:], EPS)

        # ---- initial att_value = pred @ ds_W + ds_b ----
        ps = pa.tile([P, HD], F32, tag="pa")
        nc.tensor.matmul(ps[:], predt[:, 0, :], dsw[:, 0, :], start=True, stop=False)
        nc.tensor.matmul(ps[:], predt[:, 1, :], dsw[:, 1, :], start=False, stop=True)
        av = ap.tile([P, HD], F32, tag="av")
        nc.vector.tensor_tensor(out=av[:], in0=ps[:], in1=bcast[:, BC_DSB:BC_DSB + HD],
                                op=mybir.AluOpType.add)

        def chain(l, kv, h, stag):
            """One (l, kv, h) MLP chain; writes row-major output into stag."""
            c = (l * 2 + kv) * 8 + h
            w1 = wp.tile([P, 2, F], BF16, tag="w1")
            nc.sync.dma_start(w1[:], W1.ap()[c])
            w1l = wp.tile([1, F], BF16, tag="w1l")
            nc.sync.dma_start(w1l[:], W1L.ap()[c])
            w2 = wp.tile([P, 2, F], BF16, tag="w2")
            nc.sync.dma_start(w2[:], W2.ap()[c])
            w3 = wp.tile([P, 2, DK], BF16, tag="w3")
            nc.sync.dma_start(w3[:], W3.ap()[c])

            for rc in range(2):           # row chunks of 1024
                h1 = hp.tile([P, 2, 1024], BF16, tag="h")
                for ft in range(2):
                    fsl = slice(ft * P, (ft + 1) * P)
                    for nb in range(2):
                        col = rc * 1024 + nb * 512
                        hsl = slice(nb * 512, (nb + 1) * 512)
                        ps1 = pp.tile([P, 512], F32, tag="ps")
                        nc.tensor.matmul(ps1[:], w1[:, 0, fsl],
                                         xt[:, 0, col:col + 512], start=True, stop=False)
                        nc.tensor.matmul(ps1[:], w1[:, 1, fsl],
                                         xt[:, 1, col:col + 512], start=False, stop=False)
                        nc.tensor.matmul(ps1[:], w1l[:, fsl],
                                         xtu[:, col:col + 512], start=False, stop=True)
                        nc.scalar.activation(out=h1[:, ft, hsl], in_=ps1[:],
                                             func=mybir.ActivationFunctionType.Relu,
                                             bias=pbias[:, PB_B1 + 2 * c + ft:PB_B1 + 2 * c + ft + 1])
                h2 = hp.tile([P, 2, 1024], BF16, tag="h2")
                for gt in range(2):
                    gsl = slice(gt * P, (gt + 1) * P)
                    for nb in range(2):
                        hsl = slice(nb * 512, (nb + 1) * 512)
                        ps2 = pp.tile([P, 512], F32, tag="ps")
                        nc.tensor.matmul(ps2[:], w2[:, 0, gsl],
                                         h1[:, 0, hsl], start=True, stop=False)
                        nc.tensor.matmul(ps2[:], w2[:, 1, gsl],
                                         h1[:, 1, hsl], start=False, stop=True)
                        nc.scalar.activation(out=h2[:, gt, hsl], in_=ps2[:],
                                             func=mybir.ActivationFunctionType.Relu,
                                             bias=pbias[:, PB_B2 + 2 * c + gt:PB_B2 + 2 * c + gt + 1])
                # final 256 -> 64, row-major: out[row, d]
                ps3 = pp.tile([P, 512], F32, tag="ps")
                for rti in range(8):
                    dsl = slice(rti * DK, (rti + 1) * DK)
                    rsl = slice(rti * P, (rti + 1) * P)
                    nc.tensor.matmul(ps3[:, dsl], h2[:, 0, rsl], w3[:, 0, :], start=True, stop=False)
                    nc.tensor.matmul(ps3[:, dsl], h2[:, 1, rsl], w3[:, 1, :], start=False, stop=True)
                nc.vector.tensor_copy(
                    out=stag[:, rc * 8:(rc + 1) * 8, h * DK:(h + 1) * DK],
                    in_=ps3[:].rearrange("p (r d) -> p r d", d=DK))

        def layer_norm(src, gcol, bcol, out):
            stats = ap.tile([P, 6], F32, tag="lnstat")
            nc.vector.bn_stats(stats[:], src[:])
            mv = ap.tile([P, 2], F32, tag="lnmv")
            nc.vector.bn_aggr(mv[:], stats[:])
            rstd = ap.tile([P, 1], F32, tag="lnrstd")
            nc.scalar.activation(out=rstd[:], in_=mv[:, 1:2],
                                 func=mybir.ActivationFunctionType.Sqrt, bias=epst[:, 0:1])
            nc.vector.reciprocal(rstd[:], rstd[:])
            nc.vector.tensor_scalar(out=out[:], in0=src[:], scalar1=mv[:, 0:1],
                                    scalar2=rstd[:, 0:1],
                                    op0=mybir.AluOpType.subtract,
                                    op1=mybir.AluOpType.mult)
            nc.vector.tensor_tensor(out=out[:], in0=out[:],
                                    in1=bcast[:, gcol:gcol + HD],
                                    op=mybir.AluOpType.mult)
            nc.vector.tensor_tensor(out=out[:], in0=out[:],
                                    in1=bcast[:, bcol:bcol + HD],
                                    op=mybir.AluOpType.add)

        def transpose_to(src, dst):
            """src [128, 512] f32 row-major -> dst [128, 4, 128] f32r feature-major."""
            for kt in range(4):
                tp_ = pa.tile([P, P], F32, tag="pa")
                nc.tensor.transpose(tp_[:], src[:, kt * P:(kt + 1) * P], ident[:])
                nc.scalar.activation(out=dst[:, kt, :], in_=tp_[:],
                                     func=mybir.ActivationFunctionType.Copy)

        scr = cp.tile([P, 4, HD], BF16)     # attention scratch [128, 2048]

        def attn_phase1(l, av_in, wall):
            """Gather K(l), compute logits and sigmoid pair-weights into wall."""
            kgall = gp.tile([P, 16, HD], KV_DT, tag="kgall")
            for j in range(16):
                nc.gpsimd.indirect_dma_start(
                    out=kgall[:, j, :], out_offset=None, in_=KVD[l][0].ap(),
                    in_offset=bass.IndirectOffsetOnAxis(ap=idx[:, j:j + 1], axis=0))
            logits = ap.tile([P, 16, 8], F32, tag="logits")
            avb = av_in[:, None, :].to_broadcast([P, 4, HD])
            for js in range(4):
                jsl = slice(js * 4, (js + 1) * 4)
                nc.vector.tensor_tensor(out=scr[:], in0=kgall[:, jsl, :], in1=avb,
                                        op=mybir.AluOpType.mult)
                nc.vector.tensor_reduce(
                    out=logits[:, jsl, :],
                    in_=scr[:].rearrange("p a (h d) -> p (a h) d", d=DK),
                    axis=mybir.AxisListType.X, op=mybir.AluOpType.add)
            # delta[m, s, h] = l0 - l1 ; w0 = sigmoid(scale*delta), w1 = 1 - w0
            delta = ap.tile([P, 8, 8], F32, tag="delta")
            nc.vector.tensor_tensor(out=delta[:], in0=logits[:, 0:8, :],
                                    in1=logits[:, 8:16, :],
                                    op=mybir.AluOpType.subtract)
            scale = DK ** (-0.5)
            dflat = delta[:].rearrange("p a b -> p (a b)")
            nc.scalar.activation(out=wall[:, 0:64], in_=dflat,
                                 func=mybir.ActivationFunctionType.Sigmoid, scale=scale)
            nc.scalar.activation(out=wall[:, 64:128], in_=dflat,
                                 func=mybir.ActivationFunctionType.Sigmoid, scale=-scale)

        def attn_phase2(l, av_in, wall):
            """Gather V(l), weighted-sum -> att; LN; FF; returns next av."""
            vgall = gp.tile([P, 16, HD], KV_DT, tag="vgall")
            for j in range(16):
                nc.gpsimd.indirect_dma_start(
                    out=vgall[:, j, :], out_offset=None, in_=KVD[l][1].ap(),
                    in_offset=bass.IndirectOffsetOnAxis(ap=idx[:, j:j + 1], axis=0))
            att = ap.tile([P, HD], F32, tag="att")
            wv = wall[:].rearrange("p (j h) -> p j h", h=8)
            part = ap.tile([P, HD], F32, tag="part")
            for js in range(4):
                jsl = slice(js * 4, (js + 1) * 4)
                nc.vector.tensor_tensor(
                    out=scr[:].rearrange("p a (h d) -> p a h d", d=DK),
                    in0=vgall[:, jsl, :].rearrange("p a (h d) -> p a h d", d=DK),
                    in1=wv[:, jsl, :, None].to_broadcast([P, 4, 8, DK]),
                    op=mybir.AluOpType.mult)
                dst = att if js == 0 else part
                nc.vector.tensor_reduce(
                    out=dst[:],
                    in_=scr[:].rearrange("p a hd -> p hd a"),
                    axis=mybir.AxisListType.X, op=mybir.AluOpType.add)
                if js > 0:
                    nc.vector.tensor_tensor(out=att[:], in0=att[:], in1=part[:],
                                            op=mybir.AluOpType.add)
            nc.vector.tensor_tensor(out=att[:], in0=att[:],
                                    in1=bcast[:, BC_B3V8 + l * HD:BC_B3V8 + (l + 1) * HD],
                                    op=mybir.AluOpType.add)
            nc.vector.tensor_tensor(out=att[:], in0=att[:], in1=av_in[:],
                                    op=mybir.AluOpType.add)
            xn = ap.tile([P, HD], F32, tag="xn")
            layer_norm(att, BC_LN1G + l * HD, BC_LN1B + l * HD, xn)
            # FF
            xT = ap.tile([P, 4, P], F32R, tag="xT")
            transpose_to(xn, xT)
            ffw1 = fp.tile([P, 4, HD], F32R, tag="ffw1")
            nc.sync.dma_start(ffw1[:], FFW1.ap()[l])
            ffw2 = fp.tile([P, 4, HD], F32R, tag="ffw2")
            nc.sync.dma_start(ffw2[:], FFW2.ap()[l])
            ff1 = ap.tile([P, 4, P], F32R, tag="ff1")
            for ft in range(4):
                psf = pa.tile([P, P], F32, tag="pa")
                for kt in range(4):
                    nc.tensor.matmul(psf[:], ffw1[:, kt, ft * P:(ft + 1) * P],
                                     xT[:, kt, :], start=(kt == 0), stop=(kt == 3))
                nc.scalar.activation(
                    out=ff1[:, ft, :], in_=psf[:],
                    func=mybir.ActivationFunctionType.Relu,
                    bias=pbias[:, PB_FFB1 + 4 * l + ft:PB_FFB1 + 4 * l + ft + 1])
            ps2 = pa.tile([P, HD], F32, tag="pa")
            for kt in range(4):
                nc.tensor.matmul(ps2[:], ff1[:, kt, :], ffw2[:, kt, :],
                                 start=(kt == 0), stop=(kt == 3))
            ffx = ap.tile([P, HD], F32, tag="ffx")
            nc.vector.tensor_tensor(out=ffx[:], in0=ps2[:],
                                    in1=bcast[:, BC_FFB2 + l * HD:BC_FFB2 + (l + 1) * HD],
                                    op=mybir.AluOpType.add)
            nc.vector.tensor_tensor(out=ffx[:], in0=ffx[:], in1=xn[:],
                                    op=mybir.AluOpType.add)
            av_out = ap.tile([P, HD], F32, tag="av")
            layer_norm(ffx, BC_LN2G + l * HD, BC_LN2B + l * HD, av_out)
            return av_out

        # ---- stage A + per-layer attention (phases interleaved) ----
        for l in range(L):
            wall = ap.tile([P, 128], F32, tag="wall")
            for kv in range(2):
                stag = sp.tile([P, 16, HD], KV_DT, tag="stag")
                for h in range(H):
                    chain(l, kv, h, stag)
                nc.sync.dma_start(
                    KVD[l][kv].ap().rearrange("(rt p) hd -> p rt hd", p=P),
                    stag[:])
                if kv == 0:
                    attn_phase1(l, av, wall)
            av = attn_phase2(l, av, wall)

        # ---- dist extractor ----
        avT = ap.tile([P, 4, P], F32R, tag="avT")
        transpose_to(av, avT)
        h1 = ap.tile([P, 2, P], F32R, tag="deh1")
        for ft in range(2):
            psd = pa.tile([P, P], F32, tag="pa")
            for kt in range(4):
                nc.tensor.matmul(psd[:], dew1[:, kt, ft * P:(ft + 1) * P],
                                 avT[:, kt, :], start=(kt == 0), stop=(kt == 3))
            nc.scalar.activation(out=h1[:, ft, :], in_=psd[:],
                                 func=mybir.ActivationFunctionType.Relu,
                                 bias=pbias[:, PB_DEB1 + ft:PB_DEB1 + ft + 1])
        h2 = ap.tile([P, 2, P], F32R, tag="deh2")
        for ft in range(2):
            psd = pa.tile([P, P], F32, tag="pa")
            for kt in range(2):
                nc.tensor.matmul(psd[:], dew2[:, kt, ft * P:(ft + 1) * P],
                                 h1[:, kt, :], start=(kt == 0), stop=(kt == 1))
            nc.scalar.activation(out=h2[:, ft, :], in_=psd[:],
                                 func=mybir.ActivationFunctionType.Relu,
                                 bias=pbias[:, PB_DEB2 + ft:PB_DEB2 + ft + 1])
        pso = pa.tile([P, R], F32, tag="pa")
        for kt in range(2):
            nc.tensor.matmul(pso[:], h2[:, kt, :], dew3[:, kt, :],
                             start=(kt == 0), stop=(kt == 1))
        o = ap.tile([P, R], F32, tag="out")
        nc.vector.tensor_tensor(out=o[:], in0=pso[:],
                                in1=bcast[:, BC_DEB3:BC_DEB3 + R],
                                op=mybir.AluOpType.add)
        nc.sync.dma_start(OUT.ap(), o[:])


_BUILD_CACHE = {}


def _build():
    if "nc" in _BUILD_CACHE:
        return _BUILD_CACHE["nc"]
    nc = bacc.Bacc("TRN2", target_bir_lowering=False, debug=False)
    t = []
    t.append(nc.dram_tensor("XT", [P, 2, NROW], BF16, kind="ExternalInput"))
    t.append(nc.dram_tensor("XTU", [1, NROW], BF16, kind="ExternalInput"))
    t.append(nc.dram_tensor("PREDT", [P, 2, M], F32R, kind="ExternalInput"))
    t.append(nc.dram_tensor("DSW", [P, 2, HD], F32R, kind="ExternalInput"))
    t.append(nc.dram_tensor("W1", [NCHAIN, P, 2, F], BF16, kind="ExternalInput"))
    t.append(nc.dram_tensor("W1L", [NCHAIN, 1, F], BF16, kind="ExternalInput"))
    t.append(nc.dram_tensor("W2", [NCHAIN, P, 2, F], BF16, kind="ExternalInput"))
    t.append(nc.dram_tensor("W3", [NCHAIN, P, 2, DK], BF16, kind="ExternalInput"))
    t.append(nc.dram_tensor("FFW1", [L, P, 4, HD], F32R, kind="ExternalInput"))
    t.append(nc.dram_tensor("FFW2", [L, P, 4, HD], F32R, kind="ExternalInput"))
    t.append(nc.dram_tensor("DEW1", [P, 4, F], F32R, kind="ExternalInput"))
    t.append(nc.dram_tensor("DEW2", [P, 2, F], F32R, kind="ExternalInput"))
    t.append(nc.dram_tensor("DEW3", [P, 2, R], F32R, kind="ExternalInput"))
    t.append(nc.dram_tensor("PBIAS", [P, PB_COLS], F32, kind="ExternalInput"))
    t.append(nc.dram_tensor("BCAST", [P, BC_COLS], F32, kind="ExternalInput"))
    t.append(nc.dram_tensor("IDX", [P, 16], I32, kind="ExternalInput"))
    t.append(nc.dram_tensor("OUT", [M, R], F32, kind="ExternalOutput"))
    t.append(nc.dram_tensor("KD0", [NROW, HD], KV_DT))
    t.append(nc.dram_tensor("VD0", [NROW, HD], KV_DT))
    t.append(nc.dram_tensor("KD1", [NROW, HD], KV_DT))
    t.append(nc.dram_tensor("VD1", [NROW, HD], KV_DT))
    with tile.TileContext(nc) as tc:
        _emit(nc, tc, t)
    nc.compile()
    _BUILD_CACHE["nc"] = nc
    return nc


def _prep_shared(ins):
    """Pack weights/biases/indices (identical across cores)."""
    f32 = np.float32
    kW1, kW2, kW3 = ins["kW1"], ins["kW2"], ins["kW3"]
    vW1, vW2, vW3 = ins["vW1"], ins["vW2"], ins["vW3"]
    kb1, kb2, kb3 = ins["kb1"], ins["kb2"], ins["kb3"]
    vb1, vb2, vb3 = ins["vb1"], ins["vb2"], ins["vb3"]

    W1 = np.empty((NCHAIN, P, 2, F), ml_dtypes.bfloat16)
    W1L = np.empty((NCHAIN, 1, F), ml_dtypes.bfloat16)
    W2 = np.empty((NCHAIN, P, 2, F), ml_dtypes.bfloat16)
    W3 = np.empty((NCHAIN, P, 2, DK), ml_dtypes.bfloat16)
    PB = np.zeros((P, PB_COLS), f32)
    BC = np.zeros((BC_COLS,), f32)

    for l in range(L):
        for kv in range(2):
            for h in range(H):
                c = (l * 2 + kv) * 8 + h
                w1, w2, w3 = (kW1, kW2, kW3) if kv == 0 else (vW1, vW2, vW3)
                b1, b2, b3 = (kb1, kb2, kb3) if kv == 0 else (vb1, vb2, vb3)
                W1[c] = np.asarray(w1[l, h][:256].reshape(2, P, F), f32).transpose(1, 0, 2).astype(ml_dtypes.bfloat16)
                W1L[c, 0] = np.asarray(w1[l, h][256], f32).astype(ml_dtypes.bfloat16)
                W2[c] = np.asarray(w2[l, h].reshape(2, P, F), f32).transpose(1, 0, 2).astype(ml_dtypes.bfloat16)
                W3[c] = np.asarray(w3[l, h].reshape(2, P, DK), f32).transpose(1, 0, 2).astype(ml_dtypes.bfloat16)
                for ft in range(2):
                    PB[:, PB_B1 + 2 * c + ft] = np.asarray(b1[l, h][ft * P:(ft + 1) * P], f32)
                    PB[:, PB_B2 + 2 * c + ft] = np.asarray(b2[l, h][ft * P:(ft + 1) * P], f32)
                    PB[:, PB_W1L + 2 * c + ft] = np.asarray(w1[l, h][256, ft * P:(ft + 1) * P], f32)

    DSW = np.asarray(ins["ds_W"], f32).reshape(2, P, HD).transpose(1, 0, 2).copy()
    BC[BC_DSB:BC_DSB + HD] = np.asarray(ins["ds_b"], f32)
    for l in range(L):
        # V-chain output bias folded through softmax: sum_(s,n) w*b3v = S*b3v
        BC[BC_B3V8 + l * HD:BC_B3V8 + (l + 1) * HD] = \
            S * np.asarray(vb3[l], f32).reshape(HD)

    FFW1 = np.empty((L, P, 4, HD), f32)
    FFW2 = np.empty((L, P, 4, HD), f32)
    for l in range(L):
        FFW1[l] = np.asarray(ins["ffW1"][l], f32).reshape(4, P, HD).transpose(1, 0, 2)
        FFW2[l] = np.asarray(ins["ffW2"][l], f32).reshape(4, P, HD).transpose(1, 0, 2)
        for ft in range(4):
            PB[:, PB_FFB1 + 4 * l + ft] = np.asarray(ins["ffb1"][l][ft * P:(ft + 1) * P], f32)
        BC[BC_FFB2 + l * HD:BC_FFB2 + (l + 1) * HD] = np.asarray(ins["ffb2"][l], f32)
        BC[BC_LN1G + l * HD:BC_LN1G + (l + 1) * HD] = np.asarray(ins["ln1_g"][l], f32)
        BC[BC_LN1B + l * HD:BC_LN1B + (l + 1) * HD] = np.asarray(ins["ln1_b"][l], f32)
        BC[BC_LN2G + l * HD:BC_LN2G + (l + 1) * HD] = np.asarray(ins["ln2_g"][l], f32)
        BC[BC_LN2B + l * HD:BC_LN2B + (l + 1) * HD] = np.asarray(ins["ln2_b"][l], f32)

    DEW1 = np.asarray(ins["deW1"], f32).reshape(4, P, F).transpose(1, 0, 2).copy()
    DEW2 = np.asarray(ins["deW2"], f32).reshape(2, P, F).transpose(1, 0, 2).copy()
    DEW3 = np.asarray(ins["deW3"], f32).reshape(2, P, R).transpose(1, 0, 2).copy()
    for ft in range(2):
        PB[:, PB_DEB1 + ft] = np.asarray(ins["deb1"][ft * P:(ft + 1) * P], f32)
        PB[:, PB_DEB2 + ft] = np.asarray(ins["deb2"][ft * P:(ft + 1) * P], f32)
    BC[BC_DEB3:BC_DEB3 + R] = np.asarray(ins["deb3"], f32)

    BCAST = np.broadcast_to(BC, (P, BC_COLS)).copy()

    i = int(ins["i"])
    left = np.asarray(ins["left_idx"], np.int64)
    right = np.asarray(ins["right_idx"], np.int64)
    m_ar = np.arange(M, dtype=np.int64)
    IDX = np.empty((P, 16), np.int32)
    for s in range(S):
        IDX[:, 0 * 8 + s] = (s * T + left).astype(np.int32)          # n = 0
        t1 = m_ar if s < i else right
        IDX[:, 1 * 8 + s] = (s * T + t1).astype(np.int32)            # n = 1

    return {
        "W1": W1, "W1L": W1L, "W2": W2, "W3": W3, "DSW": DSW,
        "FFW1": FFW1, "FFW2": FFW2, "DEW1": DEW1, "DEW2": DEW2, "DEW3": DEW3,
        "PBIAS": PB, "BCAST": BCAST, "IDX": IDX,
    }


import ml_dtypes as _mld


def make_in_maps(ins):
    shared = _prep_shared(ins)
    enc = np.asarray(ins["encoded"], np.float32)        # [B, S, T, D]
    tu = np.asarray(ins["true_u"], np.float32)          # [B, S, T]
    mid = np.asarray(ins["mid_idx"], np.int64)
    i = int(ins["i"])

    in_maps = []
    for b in range(B):
        x2 = enc[b].reshape(NROW, D)                    # rows (s,t), feats
        xt = x2.T.reshape(2, P, NROW).transpose(1, 0, 2).astype(ml_dtypes.bfloat16)
        pred = enc[b, i][mid]                           # [M, D]
        predt = pred.T.reshape(2, P, M).transpose(1, 0, 2).copy()
        m = dict(shared)
        m["XT"] = xt
        m["XTU"] = tu[b].reshape(1, NROW).astype(_mld.bfloat16)
        m["PREDT"] = predt
        in_maps.append(m)
    return in_maps


def kernel(**inputs):
    import os
    os.environ.setdefault("BASS_NEVER_TRACE", "1")      # keep grading runs lean
    ins = {k: np.asarray(v) for k, v in inputs.items()}
    in_maps = make_in_maps(ins)
    nc = _build()
    res = run_bass_kernel_spmd(nc, in_maps, core_ids=list(range(NCORES)))
    out = np.stack([res.results[c]["OUT"] for c in range(NCORES)])
    return out.astype(np.float32)                       # [B, M, R]


# revision 29
# speedup vs baseline: 1.0121x; 1.0121x over previous
"""Trainium2 Bass kernel for nn_AttentionalCopula (sparse_attention).

Sharding: data-parallel over batch (B=8 -> 8 cores); per-head K/V MLP stacks
computed locally per core (each core owns one batch, all heads), so no
collectives are needed.  Weights are replicated.

Per-core pipeline:
  stage A: 32 (l, kv, h) MLP chains  [2048,257] -> 256 -> 256 -> 64
           computed with transposed activations (features on partitions) and
           a row-major final projection written via staging -> DRAM.
  gather:  indirect-DMA row gathers of K/V at (s, t(m,s,n)) positions.
  attn:    DVE/ACT vector math (m on partitions), softmax-over-2 == sigmoid.
  FF/DE:   small matmuls with PE transposes at the LN -> FF boundaries.
"""

from contextlib import ExitStack

import ml_dtypes
import numpy as np

import concourse.bass as bass
import concourse.mybir as mybir
import concourse.tile as tile
from concourse import bacc
from concourse.bass_utils import run_bass_kernel_spmd
from concourse.masks import make_identity

# problem constants (hardcoded per harness contract)
B, S, T, D = 8, 8, 256, 256
H, DK = 8, 64
HD = H * DK            # 512
L = 2
F = 256
R = 512
M = 128
EPS = 1e-5
NROW = S * T           # 2048
NCORES = 8
P = 128

F32 = mybir.dt.float32
F32R = mybir.dt.float32r
BF16 = mybir.dt.bfloat16
I32 = mybir.dt.int32

KV_DT = BF16           # staging / DRAM K/V / gathered tiles

NCHAIN = 2 * L * H     # 32 chains: c = (l*2 + kv)*8 + h

# BCAST column layout (free-dim biases / LN params, replicated across partitions)
BC_DSB = 0                      # ds_b                  [512]
BC_B3V8 = 512                   # S * b3 of V chains, per l   [2*512]
BC_FFB2 = BC_B3V8 + L * HD      # ffb2 per l            [2*512]
BC_DEB3 = BC_FFB2 + L * HD      # deb3                  [512]
BC_LN1G = BC_DEB3 + R           # ln1_g per l           [2*512]
BC_LN1B = BC_LN1G + L * HD
BC_LN2G = BC_LN1B + L * HD
BC_LN2B = BC_LN2G + L * HD
BC_COLS = BC_LN2B + L * HD      # total

# PBIAS column layout (per-partition biases)
PB_B1 = 0                       # b1: 2 cols per chain (f-tile)    [64]
PB_B2 = PB_B1 + 2 * NCHAIN      # b2: 2 cols per chain             [64]
PB_FFB1 = PB_B2 + 2 * NCHAIN    # ffb1: 4 cols per l               [8]
PB_DEB1 = PB_FFB1 + 4 * L       # deb1: 2 cols                     [2]
PB_DEB2 = PB_DEB1 + 2           # deb2: 2 cols                     [2]
PB_W1L = PB_DEB2 + 2            # w1 last row (u weight): 2 cols per chain [64]
PB_COLS = PB_W1L + 2 * NCHAIN


def _emit(nc, tc, tensors):
    XT, XTU, PREDT, DSW, W1, W1L, W2, W3, FFW1, FFW2, DEW1, DEW2, DEW3, \
        PBIAS, BCAST, IDX, OUT, KD0, VD0, KD1, VD1 = tensors
    KVD = [[KD0, VD0], [KD1, VD1]]

    with ExitStack() as ctx:
        cp = ctx.enter_context(tc.tile_pool(name="const", bufs=1))
        wp = ctx.enter_context(tc.tile_pool(name="w", bufs=2))
        hp = ctx.enter_context(tc.tile_pool(name="h", bufs=3))
        sp = ctx.enter_context(tc.tile_pool(name="stag", bufs=1))
        fp = ctx.enter_context(tc.tile_pool(name="ffw", bufs=1))
        gp = ctx.enter_context(tc.tile_pool(name="gath", bufs=1))
        ap = ctx.enter_context(tc.tile_pool(name="attn", bufs=1))
        pp = ctx.enter_context(tc.tile_pool(name="ps", bufs=6, space="PSUM"))
        pa = ctx.enter_context(tc.tile_pool(name="psa", bufs=2, space="PSUM"))

        # ---- resident loads (ds-matmul inputs first) ----
        predt = cp.tile([P, 2, M], F32R)
        nc.sync.dma_start(predt[:], PREDT.ap())
        dsw = cp.tile([P, 2, HD], F32R)
        nc.sync.dma_start(dsw[:], DSW.ap())
        xt = cp.tile([P, 2, NROW], BF16)
        for cc in range(4):
            csl = slice(cc * 512, (cc + 1) * 512)
            nc.sync.dma_start(xt[:, :, csl], XT.ap()[:, :, csl])
        xtu = cp.tile([1, NROW], BF16)
        nc.sync.dma_start(xtu[:], XTU.ap())
        dew1 = cp.tile([P, 4, F], F32R)
        nc.gpsimd.dma_start(dew1[:], DEW1.ap())
        dew2 = cp.tile([P, 2, F], F32R)
        nc.gpsimd.dma_start(dew2[:], DEW2.ap())
        dew3 = cp.tile([P, 2, R], F32R)
        nc.gpsimd.dma_start(dew3[:], DEW3.ap())
        pbias = cp.tile([P, PB_COLS], F32)
        nc.sync.dma_start(pbias[:], PBIAS.ap())
        bcast = cp.tile([P, BC_COLS], F32)
        nc.gpsimd.dma_start(bcast[:], BCAST.ap())
        idx = cp.tile([P, 16], I32)
        nc.sync.dma_start(idx[:], IDX.ap())
        ident = cp.tile([P, P], F32)
        make_identity(nc, ident[:])
        epst = cp.tile([P, 1], F32)
        nc.vector.memset(epst[:], EPS)

        # ---- initial att_value = pred @ ds_W + ds_b ----
        ps = pa.tile([P, HD], F32, tag="pa")
        nc.tensor.matmul(ps[:], predt[:, 0, :], dsw[:, 0, :], start=True, stop=False)
        nc.tensor.matmul(ps[:], predt[:, 1, :], dsw[:, 1, :], start=False, stop=True)
        av = ap.tile([P, HD], F32, tag="av")
        nc.vector.tensor_tensor(out=av[:], in0=ps[:], in1=bcast[:, BC_DSB:BC_DSB + HD],
                                op=mybir.AluOpType.add)

        def chain(l, kv, h, stag):
            """One (l, kv, h) MLP chain; writes row-major output into stag."""
            c = (l * 2 + kv) * 8 + h
            w1 = wp.tile([P, 2, F], BF16, tag="w1")
            nc.sync.dma_start(w1[:], W1.ap()[c])
            w1l = wp.tile([1, F], BF16, tag="w1l")
            nc.sync.dma_start(w1l[:], W1L.ap()[c])
            w2 = wp.tile([P, 2, F], BF16, tag="w2")
            nc.sync.dma_start(w2[:], W2.ap()[c])
            w3 = wp.tile([P, 2, DK], BF16, tag="w3")
            nc.sync.dma_start(w3[:], W3.ap()[c])

            for rc in range(2):           # row chunks of 1024
                h1 = hp.tile([P, 2, 1024], BF16, tag="h")
                for ft in range(2):
                    fsl = slice(ft * P, (ft + 1) * P)
                    for nb in range(2):
                        col = rc * 1024 + nb * 512
                        hsl = slice(nb * 512, (nb + 1) * 512)
                        ps1 = pp.tile([P, 512], F32, tag="ps")
                        nc.tensor.matmul(ps1[:], w1[:, 0, fsl],
                                         xt[:, 0, col:col + 512], start=True, stop=False)
                        nc.tensor.matmul(ps1[:], w1[:, 1, fsl],
                                         xt[:, 1, col:col + 512], start=False, stop=False)
                        nc.tensor.matmul(ps1[:], w1l[:, fsl],
                                         xtu[:, col:col + 512], start=False, stop=True)
                        nc.scalar.activation(out=h1[:, ft, hsl], in_=ps1[:],
                                             func=mybir.ActivationFunctionType.Relu,
                                             bias=pbias[:, PB_B1 + 2 * c + ft:PB_B1 + 2 * c + ft + 1])
                h2 = hp.tile([P, 2, 1024], BF16, tag="h2")
                for gt in range(2):
                    gsl = slice(gt * P, (gt + 1) * P)
                    for nb in range(2):
                        hsl = slice(nb * 512, (nb + 1) * 512)
                        ps2 = pp.tile([P, 512], F32, tag="ps")
                        nc.tensor.matmul(ps2[:], w2[:, 0, gsl],
                                         h1[:, 0, hsl], start=True, stop=False)
                        nc.tensor.matmul(ps2[:], w2[:, 1, gsl],
                                         h1[:, 1, hsl], start=False, stop=True)
                        nc.scalar.activation(out=h2[:, gt, hsl], in_=ps2[:],
                                             func=mybir.ActivationFunctionType.Relu,
                                             bias=pbias[:, PB_B2 + 2 * c + gt:PB_B2 + 2 * c + gt + 1])
                # final 256 -> 64, row-major: out[row, d]
                ps3 = pp.tile([P, 512], F32, tag="ps")
                for rti in range(8):
                    dsl = slice(rti * DK, (rti + 1) * DK)
                    rsl = slice(rti * P, (rti + 1) * P)
                    nc.tensor.matmul(ps3[:, dsl], h2[:, 0, rsl], w3[:, 0, :], start=True, stop=False)
                    nc.tensor.matmul(ps3[:, dsl], h2[:, 1, rsl], w3[:, 1, :], start=False, stop=True)
                nc.vector.tensor_copy(
                    out=stag[:, rc * 8:(rc + 1) * 8, h * DK:(h + 1) * DK],
                    in_=ps3[:].rearrange("p (r d) -> p r d", d=DK))

        def layer_norm(src, gcol, bcol, out):
            stats = ap.tile([P, 6], F32, tag="lnstat")
            nc.vector.bn_stats(stats[:], src[:])
            mv = ap.tile([P, 2], F32, tag="lnmv")
            nc.vector.bn_aggr(mv[:], stats[:])
            rstd = ap.tile([P, 1], F32, tag="lnrstd")
            nc.scalar.activation(out=rstd[:], in_=mv[:, 1:2],
                                 func=mybir.ActivationFunctionType.Sqrt, bias=epst[:, 0:1])
            nc.vector.reciprocal(rstd[:], rstd[:])
            nc.vector.tensor_scalar(out=out[:], in0=src[:], scalar1=mv[:, 0:1],
                                    scalar2=rstd[:, 0:1],
                                    op0=mybir.AluOpType.subtract,
                                    op1=mybir.AluOpType.mult)
            nc.vector.tensor_tensor(out=out[:], in0=out[:],
                                    in1=bcast[:, gcol:gcol + HD],
                                    op=mybir.AluOpType.mult)
            nc.vector.tensor_tensor(out=out[:], in0=out[:],
                                    in1=bcast[:, bcol:bcol + HD],
                                    op=mybir.AluOpType.add)

        def transpose_to(src, dst):
            """src [128, 512] f32 row-major -> dst [128, 4, 128] f32r feature-major."""
            for kt in range(4):
                tp_ = pa.tile([P, P], F32, tag="pa")
                nc.tensor.transpose(tp_[:], src[:, kt * P:(kt + 1) * P], ident[:])
                nc.scalar.activation(out=dst[:, kt, :], in_=tp_[:],
                                     func=mybir.ActivationFunctionType.Copy)

        scr = cp.tile([P, 4, HD], BF16)     # attention scratch [128, 2048]

        def attn_phase1(l, av_in, wall):
            """Gather K(l), compute logits and sigmoid pair-weights into wall."""
            kgall = gp.tile([P, 16, HD], KV_DT, tag="kgall")
            for j in range(16):
                nc.gpsimd.indirect_dma_start(
                    out=kgall[:, j, :], out_offset=None, in_=KVD[l][0].ap(),
                    in_offset=bass.IndirectOffsetOnAxis(ap=idx[:, j:j + 1], axis=0))
            logits = ap.tile([P, 16, 8], F32, tag="logits")
            avb = av_in[:, None, :].to_broadcast([P, 4, HD])
            for js in range(4):
                jsl = slice(js * 4, (js + 1) * 4)
                nc.vector.tensor_tensor(out=scr[:], in0=kgall[:, jsl, :], in1=avb,
                                        op=mybir.AluOpType.mult)
                nc.vector.tensor_reduce(
                    out=logits[:, jsl, :],
                    in_=scr[:].rearrange("p a (h d) -> p (a h) d", d=DK),
                    axis=mybir.AxisListType.X, op=mybir.AluOpType.add)
            # delta[m, s, h] = l0 - l1 ; w0 = sigmoid(scale*delta), w1 = 1 - w0
            delta = ap.tile([P, 8, 8], F32, tag="delta")
            nc.vector.tensor_tensor(out=delta[:], in0=logits[:, 0:8, :],
                                    in1=logits[:, 8:16, :],
                                    op=mybir.AluOpType.subtract)
            scale = DK ** (-0.5)
            dflat = delta[:].rearrange("p a b -> p (a b)")
            nc.scalar.activation(out=wall[:, 0:64], in_=dflat,
                                 func=mybir.ActivationFunctionType.Sigmoid, scale=scale)
            nc.scalar.activation(out=wall[:, 64:128], in_=dflat,
                                 func=mybir.ActivationFunctionType.Sigmoid, scale=-scale)

        def attn_phase2(l, av_in, wall):
            """Gather V(l), weighted-sum -> att; LN; FF; returns next av."""
            vgall = gp.tile([P, 16, HD], KV_DT, tag="vgall")
            for j in range(16):
                nc.gpsimd.indirect_dma_start(
                    out=vgall[:, j, :], out_offset=None, in_=KVD[l][1].ap(),
                    in_offset=bass.IndirectOffsetOnAxis(ap=idx[:, j:j + 1], axis=0))
            att = ap.tile([P, HD], F32, tag="att")
            wv = wall[:].rearrange("p (j h) -> p j h", h=8)
            part = ap.tile([P, HD], F32, tag="part")
            for js in range(4):
                jsl = slice(js * 4, (js + 1) * 4)
                nc.vector.tensor_tensor(
                    out=scr[:].rearrange("p a (h d) -> p a h d", d=DK),
                    in0=vgall[:, jsl, :].rearrange("p a (h d) -> p a h d", d=DK),
                    in1=wv[:, jsl, :, None].to_broadcast([P, 4, 8, DK]),
                    op=mybir.AluOpType.mult)
                dst = att if js == 0 else part
                nc.vector.tensor_reduce(
                    out=dst[:],
                    in_=scr[:].rearrange("p a hd -> p hd a"),
                    axis=mybir.AxisListType.X, op=mybir.AluOpType.add)
                if js > 0:
                    nc.vector.tensor_tensor(out=att[:], in0=att[:], in1=part[:],
                                            op=mybir.AluOpType.add)
            nc.vector.tensor_tensor(out=att[:], in0=att[:],
                                    in1=bcast[:, BC_B3V8 + l * HD:BC_B3V8 + (l + 1) * HD],
                                    op=mybir.AluOpType.add)
            nc.vector.tensor_tensor(out=att[:], in0=att[:], in1=av_in[:],
                                    op=mybir.AluOpType.add)
            xn = ap.tile([P, HD], F32, tag="xn")
            layer_norm(att, BC_LN1G + l * HD, BC_LN1B + l * HD, xn)
            # FF
            xT = ap.tile([P, 4, P], F32R, tag="xT")
            transpose_to(xn, xT)
            ffw1 = fp.tile([P, 4, HD], F32R, tag="ffw1")
            nc.sync.dma_start(ffw1[:], FFW1.ap()[l])
            ffw2 = fp.tile([P, 4, HD], F32R, tag="ffw2")
            nc.sync.dma_start(ffw2[:], FFW2.ap()[l])
            ff1 = ap.tile([P, 4, P], F32R, tag="ff1")
            for ft in range(4):
                psf = pa.tile([P, P], F32, tag="pa")
                for kt in range(4):
                    nc.tensor.matmul(psf[:], ffw1[:, kt, ft * P:(ft + 1) * P],
                                     xT[:, kt, :], start=(kt == 0), stop=(kt == 3))
                nc.scalar.activation(
                    out=ff1[:, ft, :], in_=psf[:],
                    func=mybir.ActivationFunctionType.Relu,
                    bias=pbias[:, PB_FFB1 + 4 * l + ft:PB_FFB1 + 4 * l + ft + 1])
            ps2 = pa.tile([P, HD], F32, tag="pa")
            for kt in range(4):
                nc.tensor.matmul(ps2[:], ff1[:, kt, :], ffw2[:, kt, :],
                                 start=(kt == 0), stop=(kt == 3))
            ffx = ap.tile([P, HD], F32, tag="ffx")
            nc.vector.tensor_tensor(out=ffx[:], in0=ps2[:],
                                    in1=bcast[:, BC_FFB2 + l * HD:BC_FFB2 + (l + 1) * HD],
                                    op=mybir.AluOpType.add)
            nc.vector.tensor_tensor(out=ffx[:], in0=ffx[:], in1=xn[:],
                                    op=mybir.AluOpType.add)
            av_out = ap.tile([P, HD], F32, tag="av")
            layer_norm(ffx, BC_LN2G + l * HD, BC_LN2B + l * HD, av_out)
            return av_out

        # ---- stage A + per-layer attention (phases interleaved) ----
        for l in range(L):
            wall = ap.tile([P, 128], F32, tag="wall")
            for kv in range(2):
                stag = sp.tile([P, 16, HD], KV_DT, tag="stag")
                for h in range(H):
                    chain(l, kv, h, stag)
                nc.sync.dma_start(
                    KVD[l][kv].ap().rearrange("(rt p) hd -> p rt hd", p=P),
                    stag[:])
                if kv == 0:
                    attn_phase1(l, av, wall)
            av = attn_phase2(l, av, wall)

        # ---- dist extractor ----
        avT = ap.tile([P, 4, P], F32R, tag="avT")
        transpose_to(av, avT)
        h1 = ap.tile([P, 2, P], F32R, tag="deh1")
        for ft in range(2):
            psd = pa.tile([P, P], F32, tag="pa")
            for kt in range(4):
                nc.tensor.matmul(psd[:], dew1[:, kt, ft * P:(ft + 1) * P],
                                 avT[:, kt, :], start=(kt == 0), stop=(kt == 3))
            nc.scalar.activation(out=h1[:, ft, :], in_=psd[:],
                                 func=mybir.ActivationFunctionType.Relu,
                                 bias=pbias[:, PB_DEB1 + ft:PB_DEB1 + ft + 1])
        h2 = ap.tile([P, 2, P], F32R, tag="deh2")
        for ft in range(2):
            psd = pa.tile([P, P], F32, tag="pa")
            for kt in range(2):
                nc.tensor.matmul(psd[:], dew2[:, kt, ft * P:(ft + 1) * P],
                                 h1[:, kt, :], start=(kt == 0), stop=(kt == 1))
            nc.scalar.activation(out=h2[:, ft, :], in_=psd[:],
                                 func=mybir.ActivationFunctionType.Relu,
                                 bias=pbias[:, PB_DEB2 + ft:PB_DEB2 + ft + 1])
        pso = pa.tile([P, R], F32, tag="pa")
        for kt in range(2):
            nc.tensor.matmul(pso[:], h2[:, kt, :], dew3[:, kt, :],
                             start=(kt == 0), stop=(kt == 1))
        o = ap.tile([P, R], F32, tag="out")
        nc.vector.tensor_tensor(out=o[:], in0=pso[:],
                                in1=bcast[:, BC_DEB3:BC_DEB3 + R],
                                op=mybir.AluOpType.add)
        nc.sync.dma_start(OUT.ap(), o[:])


_BUILD_CACHE = {}


def _build():
    if "nc" in _BUILD_CACHE:
        return _BUILD_CACHE["nc"]
    nc = bacc.Bacc("TRN2", target_bir_lowering=False, debug=False)
    t = []
    t.append(nc.dram_tensor("XT", [P, 2, NROW], BF16, kind="ExternalInput"))
    t.append(nc.dram_tensor("XTU", [1, NROW], BF16, kind="ExternalInput"))
    t.append(nc.dram_tensor("PREDT", [P, 2, M], F32R, kind="ExternalInput"))
    t.append(nc.dram_tensor("DSW", [P, 2, HD], F32R, kind="ExternalInput"))
    t.append(nc.dram_tensor("W1", [NCHAIN, P, 2, F], BF16, kind="ExternalInput"))
    t.append(nc.dram_tensor("W1L", [NCHAIN, 1, F], BF16, kind="ExternalInput"))
    t.append(nc.dram_tensor("W2", [NCHAIN, P, 2, F], BF16, kind="ExternalInput"))
    t.append(nc.dram_tensor("W3", [NCHAIN, P, 2, DK], BF16, kind="ExternalInput"))
    t.append(nc.dram_tensor("FFW1", [L, P, 4, HD], F32R, kind="ExternalInput"))
    t.append(nc.dram_tensor("FFW2", [L, P, 4, HD], F32R, kind="ExternalInput"))
    t.append(nc.dram_tensor("DEW1", [P, 4, F], F32R, kind="ExternalInput"))
    t.append(nc.dram_tensor("DEW2", [P, 2, F], F32R, kind="ExternalInput"))
    t.append(nc.dram_tensor("DEW3", [P, 2, R], F32R, kind="ExternalInput"))
    t.append(nc.dram_tensor("PBIAS", [P, PB_COLS], F32, kind="ExternalInput"))
    t.append(nc.dram_tensor("BCAST", [P, BC_COLS], F32, kind="ExternalInput"))
    t.append(nc.dram_tensor("IDX", [P, 16], I32, kind="ExternalInput"))
    t.append(nc.dram_tensor("OUT", [M, R], F32, kind="ExternalOutput"))
    t.append(nc.dram_tensor("KD0", [NROW, HD], KV_DT))
    t.append(nc.dram_tensor("VD0", [NROW, HD], KV_DT))
    t.append(nc.dram_tensor("KD1", [NROW, HD], KV_DT))
    t.append(nc.dram_tensor("VD1", [NROW, HD], KV_DT))
    with tile.TileContext(nc) as tc:
        _emit(nc, tc, t)
    nc.compile()
    _BUILD_CACHE["nc"] = nc
    return nc


def _prep_shared(ins):
    """Pack weights/biases/indices (identical across cores)."""
    f32 = np.float32
    kW1, kW2, kW3 = ins["kW1"], ins["kW2"], ins["kW3"]
    vW1, vW2, vW3 = ins["vW1"], ins["vW2"], ins["vW3"]
    kb1, kb2, kb3 = ins["kb1"], ins["kb2"], ins["kb3"]
    vb1, vb2, vb3 = ins["vb1"], ins["vb2"], ins["vb3"]

    W1 = np.empty((NCHAIN, P, 2, F), ml_dtypes.bfloat16)
    W1L = np.empty((NCHAIN, 1, F), ml_dtypes.bfloat16)
    W2 = np.empty((NCHAIN, P, 2, F), ml_dtypes.bfloat16)
    W3 = np.empty((NCHAIN, P, 2, DK), ml_dtypes.bfloat16)
    PB = np.zeros((P, PB_COLS), f32)
    BC = np.zeros((BC_COLS,), f32)

    for l in range(L):
        for kv in range(2):
            for h in range(H):
                c = (l * 2 + kv) * 8 + h
                w1, w2, w3 = (kW1, kW2, kW3) if kv == 0 else (vW1, vW2, vW3)
                b1, b2, b3 = (kb1, kb2, kb3) if kv == 0 else (vb1, vb2, vb3)
                W1[c] = np.asarray(w1[l, h][:256].reshape(2, P, F), f32).transpose(1, 0, 2).astype(ml_dtypes.bfloat16)
                W1L[c, 0] = np.asarray(w1[l, h][256], f32).astype(ml_dtypes.bfloat16)
                W2[c] = np.asarray(w2[l, h].reshape(2, P, F), f32).transpose(1, 0, 2).astype(ml_dtypes.bfloat16)
                W3[c] = np.asarray(w3[l, h].reshape(2, P, DK), f32).transpose(1, 0, 2).astype(ml_dtypes.bfloat16)
                for ft in range(2):
                    PB[:, PB_B1 + 2 * c + ft] = np.asarray(b1[l, h][ft * P:(ft + 1) * P], f32)
                    PB[:, PB_B2 + 2 * c + ft] = np.asarray(b2[l, h][ft * P:(ft + 1) * P], f32)
                    PB[:, PB_W1L + 2 * c + ft] = np.asarray(w1[l, h][256, ft * P:(ft + 1) * P], f32)

    DSW = np.asarray(ins["ds_W"], f32).reshape(2, P, HD).transpose(1, 0, 2).copy()
    BC[BC_DSB:BC_DSB + HD] = np.asarray(ins["ds_b"], f32)
    for l in range(L):
        # V-chain output bias folded through softmax: sum_(s,n) w*b3v = S*b3v
        BC[BC_B3V8 + l * HD:BC_B3V8 + (l + 1) * HD] = \
            S * np.asarray(vb3[l], f32).reshape(HD)

    FFW1 = np.empty((L, P, 4, HD), f32)
    FFW2 = np.empty((L, P, 4, HD), f32)
    for l in range(L):
        FFW1[l] = np.asarray(ins["ffW1"][l], f32).reshape(4, P, HD).transpose(1, 0, 2)
        FFW2[l] = np.asarray(ins["ffW2"][l], f32).reshape(4, P, HD).transpose(1, 0, 2)
        for ft in range(4):
            PB[:, PB_FFB1 + 4 * l + ft] = np.asarray(ins["ffb1"][l][ft * P:(ft + 1) * P], f32)
        BC[BC_FFB2 + l * HD:BC_FFB2 + (l + 1) * HD] = np.asarray(ins["ffb2"][l], f32)
        BC[BC_LN1G + l * HD:BC_LN1G + (l + 1) * HD] = np.asarray(ins["ln1_g"][l], f32)
        BC[BC_LN1B + l * HD:BC_LN1B + (l + 1) * HD] = np.asarray(ins["ln1_b"][l], f32)
        BC[BC_LN2G + l * HD:BC_LN2G + (l + 1) * HD] = np.asarray(ins["ln2_g"][l], f32)
        BC[BC_LN2B + l * HD:BC_LN2B + (l + 1) * HD] = np.asarray(ins["ln2_b"][l], f32)

    DEW1 = np.asarray(ins["deW1"], f32).reshape(4, P, F).transpose(1, 0, 2).copy()
    DEW2 = np.asarray(ins["deW2"], f32).reshape(2, P, F).transpose(1, 0, 2).copy()
    DEW3 = np.asarray(ins["deW3"], f32).reshape(2, P, R).transpose(1, 0, 2).copy()
    for ft in range(2):
        PB[:, PB_DEB1 + ft] = np.asarray(ins["deb1"][ft * P:(ft + 1) * P], f32)
        PB[:, PB_DEB2 + ft] = np.asarray(ins["deb2"][ft * P:(ft + 1) * P], f32)
    BC[BC_DEB3:BC_DEB3 + R] = np.asarray(ins["deb3"], f32)

    BCAST = np.broadcast_to(BC, (P, BC_COLS)).copy()

    i = int(ins["i"])
    left = np.asarray(ins["left_idx"], np.int64)
    right = np.asarray(ins["right_idx"], np.int64)
    m_ar = np.arange(M, dtype=np.int64)
    IDX = np.empty((P, 16), np.int32)
    for s in range(S):
        IDX[:, 0 * 8 + s] = (s * T + left).astype(np.int32)          # n = 0
        t1 = m_ar if s < i else right
        IDX[:, 1 * 8 + s] = (s * T + t1).astype(np.int32)            # n = 1

    return {
        "W1": W1, "W1L": W1L, "W2": W2, "W3": W3, "DSW": DSW,
        "FFW1": FFW1, "FFW2": FFW2, "DEW1": DEW1, "DEW2": DEW2, "DEW3": DEW3,
        "PBIAS": PB, "BCAST": BCAST, "IDX": IDX,
    }


import ml_dtypes as _mld


def make_in_maps(ins):
    shared = _prep_shared(ins)
    enc = np.asarray(ins["encoded"], np.float32)        # [B, S, T, D]
    tu = np.asarray(ins["true_u"], np.float32)          # [B, S, T]
    mid = np.asarray(ins["mid_idx"], np.int64)
    i = int(ins["i"])

    in_maps = []
    for b in range(B):
        x2 = enc[b].reshape(NROW, D)                    # rows (s,t), feats
        xt = x2.T.reshape(2, P, NROW).transpose(1, 0, 2).astype(ml_dtypes.bfloat16)
        pred = enc[b, i][mid]                           # [M, D]
        predt = pred.T.reshape(2, P, M).transpose(1, 0, 2).copy()
        m = dict(shared)
        m["XT"] = xt
        m["XTU"] = tu[b].reshape(1, NROW).astype(_mld.bfloat16)
        m["PREDT"] = predt
        in_maps.append(m)
    return in_maps


def kernel(**inputs):
    import os
    os.environ.setdefault("BASS_NEVER_TRACE", "1")      # keep grading runs lean
    ins = {k: np.asarray(v) for k, v in inputs.items()}
    in_maps = make_in_maps(ins)
    nc = _build()
    res = run_bass_kernel_spmd(nc, in_maps, core_ids=list(range(NCORES)))
    out = np.stack([res.results[c]["OUT"] for c in range(NCORES)])
    return out.astype(np.float32)                       # [B, M, R]
